# revision 1
# baseline (speedup 1.0000x reference)
"""Per-sample dynamic conv2d (VALID) on 8 Trainium2 NeuronCores — v3.

v3 = v2 (bf16 upload, DMA-xbar transpose loads, 9-tap PSUM accumulation)
with quad row-packing: each 2KB PSUM bank holds FOUR consecutive output rows
(4 x 128 f32).  The kernel taps are host-rearranged to [kw, j=2-kh] order so
that for input row r and kw, the taps of consecutive target rows are
CONTIGUOUS 128-col blocks in SBUF: one matmul with N=128*nrows covers a whole
run of rows inside a quad (psum cols (hp-4q)*128...).  This roughly halves
the matmul instruction count (same streamed columns) and evacuates four rows
per copy instead of one.

Quad q (rows 4q..4q+3) opens at (r=4q, kw=0) with start=True and closes at
(r=4q+5, kw=2) with stop=True (q=31 holds rows 124-125, closes at r=127).
All evacuations run on DVE (the PE's start-matmuls wait on them via bank
recycling, and the DVE queue never carries DMA-lane waits); stores batch 6
quads (24 rows) as bf16 on the ACT HWDGE ring (host casts the output back to
f32), keeping the SP ring free for the transpose loads.  ot bufs=6 decouples
evacuations from store completions (which queue behind transpose traffic on
the shared SDMA engines).
"""

import numpy as np
import ml_dtypes

import concourse.bass as bass
import concourse.mybir as mybir
from concourse.bass_utils import run_bass_kernel_spmd
from concourse.tile import TileContext

N_CORES = 8
B, H, W, C = 32, 128, 128, 128
KK = 3
BL = B // N_CORES            # samples per core
HO = WO = H - KK + 1         # 126
HW = H * W
XT_PAD = HW + 128            # matmuls read up to HW+2
NQ = (HO + 3) // 4           # 32 quads (last holds 2 rows)
QG = 6                       # quads per store group (24 rows; 6 stores/sample)

F32 = mybir.dt.float32
BF16 = mybir.dt.bfloat16


def _split_excess_waits(nc, limit=1):
    """walrus codegen rejects >1 sync-wait on several instruction kinds.
    Move excess waits onto preceding same-engine NoOps."""
    n = 0
    for bb in nc.m.functions[0].blocks:
        out = []
        changed = False
        for inst in bb.instructions:
            si = inst.sync_info
            if si is not None and len(si.on_wait) > limit:
                waits = list(si.on_wait)
                excess, keep = waits[:-limit], waits[-limit:]
                for i in range(0, len(excess), limit):
                    n += 1
                    out.append(
                        mybir.InstNoOp(
                            name=f"I-waitsplit-{n}",
                            engine=inst.engine,
                            bass_nofuse=True,
                            sync_info=mybir.SyncInfo(
                                on_wait=excess[i : i + limit], on_update=[]
                            ),
                        )
                    )
                inst.sync_info = mybir.SyncInfo(on_wait=keep, on_update=si.on_update)
                changed = True
            out.append(inst)
        if changed:
            bb.instructions = out
    return n


def _build():
    nc = bass.Bass()
    Xd = nc.declare_dram_parameter("X", [BL, HW, C], BF16, isOutput=False)
    # host-rearranged: t = kw*3 + j with j = 2-kh
    Kd = nc.declare_dram_parameter("kern", [BL, KK * KK, C, C], BF16, isOutput=False)
    Od = nc.declare_dram_parameter("out", [BL, HO, WO, C], BF16, isOutput=True)

    with TileContext(nc) as tc:
        with (
            tc.tile_pool(name="xt", bufs=3) as p_xt,
            tc.tile_pool(name="kt", bufs=3) as p_k,
            tc.tile_pool(name="outb", bufs=6) as p_out,
            tc.tile_pool(name="pacc", bufs=8, space="PSUM") as p_acc,
        ):
            def emit_load(b, nchunks):
                xt = p_xt.tile([C, XT_PAD], BF16, tag="xt")
                step = HW // nchunks
                for c0 in range(0, HW, step):
                    nc.sync.dma_start(
                        out=xt[:, c0 : c0 + step],
                        in_=Xd[b, c0 : c0 + step, :],
                        transpose=True,
                    )
                # small; SWDGE ring is idle — keeps it off the transpose FIFO
                kall = p_k.tile([C, KK * KK * C], BF16, tag="kall")
                nc.gpsimd.dma_start(
                    out=kall[:, :].rearrange("ci (t co) -> ci t co", t=KK * KK),
                    in_=Kd[b].rearrange("t ci co -> ci t co"),
                )
                return {"kall": kall, "xt": xt}

            def emit_compute(b, st):
                kall, xt = st["kall"], st["xt"]
                live = {}
                ot = None
                for r in range(H):
                    for kw in range(KK):
                        x_sl = xt[:, r * 128 + kw : r * 128 + kw + 128]
                        a, hp_hi = max(0, r - 2), min(r, HO - 1)
                        while a <= hp_hi:
                            q = a // 4
                            b_end = min(hp_hi, 4 * q + 3)
                            nt = b_end - a + 1
                            if q not in live:
                                pr = p_acc.tile([W, 512], F32, tag="P")
                                live[q] = pr
                            j_a = a - r + 2
                            close_r = 4 * q + 5 if q < NQ - 1 else H - 1
                            nc.tensor.matmul(
                                live[q][:, (a - 4 * q) * C : (a - 4 * q + nt) * C],
                                x_sl,
                                kall[:, (kw * KK + j_a) * C : (kw * KK + j_a + nt) * C],
                                start=(r == 4 * q and kw == 0),
                                stop=(r == close_r and kw == KK - 1),
                            )
                            a = b_end + 1
                    # quads closing at this r
                    closed = []
                    if r >= 5 and (r - 5) % 4 == 0:
                        closed.append((r - 5) // 4)
                    if r == H - 1:
                        closed.append(NQ - 1)
                    for q in closed:
                        nrows = min(4, HO - 4 * q)
                        g = q // QG          # store group
                        k = q % QG           # quad slot within group
                        if k == 0:
                            ot = p_out.tile([WO, QG * 4 * C], BF16, tag="ot")
                        src = live.pop(q)[0:WO, 0 : nrows * C]
                        dst = ot[0:WO, k * 4 * C : (k * 4 + nrows) * C]
                        # ALL evacs on DVE: the PE's start-matmuls wait on
                        # these via bank recycling, and the DVE queue never
                        # carries DMA-lane waits (unlike ACT, whose store
                        # DMAs wait on lanes recycled from the transposes).
                        nc.vector.tensor_copy(dst, src)
                        last_in_group = (k == QG - 1) or (q == NQ - 1)
                        if last_in_group:
                            base = g * QG * 4
                            nr = k * 4 + nrows
                            nc.scalar.dma_start(
                                out=Od[b, base : base + nr].rearrange(
                                    "h w c -> w h c"
                                ),
                                in_=ot[0:WO, 0 : nr * C].rearrange(
                                    "w (h c) -> w h c", h=nr
                                ),
                            )

            # 2 loads + 6 stores per sample = 8 HWDGE DMAs, matching the 8
            # DMAHW completion-sem lanes Tile round-robins: each DMA's
            # lane-recycling wait lands on the same DMA kind one sample
            # back, so loads never gate on stores (which would make them
            # just-in-time instead of prefetched).
            st = emit_load(0, nchunks=8)
            for b in range(BL):
                nxt = emit_load(b + 1, nchunks=2) if b + 1 < BL else None
                emit_compute(b, st)
                st = nxt

    _split_excess_waits(nc)
    return nc


_CACHE = {}


def _get_nc():
    if "nc" not in _CACHE:
        _CACHE["nc"] = _build()
    return _CACHE["nc"]


def _run(X, kern, **kw):
    Xb = X.astype(ml_dtypes.bfloat16).reshape(B, HW, C)
    # [B, kh, kw, ci, co] -> [B, kw, j=2-kh, ci, co] -> [B, 9, ci, co]
    Kb = (
        kern.astype(ml_dtypes.bfloat16)[:, ::-1]
        .transpose(0, 2, 1, 3, 4)
        .reshape(B, KK * KK, C, C)
    )
    in_maps = [
        {
            "X": np.ascontiguousarray(Xb[c * BL : (c + 1) * BL]),
            "kern": np.ascontiguousarray(Kb[c * BL : (c + 1) * BL]),
        }
        for c in range(N_CORES)
    ]
    last_err = None
    for _attempt in range(3):
        try:
            res = run_bass_kernel_spmd(
                _get_nc(), in_maps, list(range(N_CORES)), **kw
            )
            break
        except Exception as e:  # transient NRT_EXEC_UNIT_UNRECOVERABLE etc.
            last_err = e
    else:
        raise last_err
    out = np.concatenate(
        [np.asarray(res.results[c]["out"]).astype(np.float32) for c in range(N_CORES)],
        axis=0,
    )
    return out, res


def kernel(X, kernel):
    X = np.ascontiguousarray(X, dtype=np.float32)
    kern = np.ascontiguousarray(kernel, dtype=np.float32)
    out, _ = _run(X, kern)
    return out



# revision 4
# speedup vs baseline: 1.2010x; 1.2010x over previous
"""Per-sample dynamic conv2d (VALID) on 8 Trainium2 NeuronCores — v4.

v4 = 1-D Winograd F(2,3) along W, direct 3-tap accumulation along H.
Cuts the PE moving-column count 1.5x vs the direct v3 kernel (145k ->
97k streamed columns per sample).

Math per sample (hp = output row, jt = W-tile, w = 2*jt + a):
  V0 = x[2j]-x[2j+2], V1 = x[2j+1]+x[2j+2],
  V2 = x[2j+2]-x[2j+1], V3 = x[2j+1]-x[2j+3]          (host, bf16)
  U[kh,xi] = G @ K[kh,:]  (G = F(2,3) kernel transform) (host, bf16)
  M[xi][hp] = sum_kh V[xi][hp+kh] @ U[kh,xi]            (PE, 12 MMs/group)
  y_even = M0+M1+M2,  y_odd = M1-M2-M3                  (ACT copies + DVE adds)

Device layout: psum/output partitions = Cout.  Output is written as
[C, 2, HO, 63] (even/odd de-interleaved) and the host transposes back —
all device DMA is therefore fully linear.

Per 8-row group x 4 xi: 3 accumulating matmuls (stationary U[kh,xi],
moving = contiguous 504-col V slice) into 4 psum banks; ACT evacuates
M0..M2 to bf16 SBUF, DVE evacuates M3 and does the 4 inverse-transform
adds.  2 groups of psum (8 banks) in flight.
"""

import numpy as np
import ml_dtypes

import concourse.bass as bass
import concourse.mybir as mybir
from concourse.bass_utils import run_bass_kernel_spmd
from concourse.tile import TileContext

N_CORES = 8
B, H, W, C = 32, 128, 128, 128
KK = 3
XI = 4                       # Winograd phases
BL = B // N_CORES            # samples per core
HO = WO = H - KK + 1         # 126
NJ = WO // 2                 # 63 W-tiles
HPG = 8                      # output rows per group
NG = (HO + HPG - 1) // HPG   # 16 groups (last holds 6 rows)
VSZ = XI * H * NJ            # 32256 free elems of V per sample

F32 = mybir.dt.float32
BF16 = mybir.dt.bfloat16


def _split_excess_waits(nc, limit=1):
    """walrus codegen rejects >1 sync-wait on several instruction kinds.
    Move excess waits onto preceding same-engine NoOps."""
    n = 0
    for bb in nc.m.functions[0].blocks:
        out = []
        changed = False
        for inst in bb.instructions:
            si = inst.sync_info
            if si is not None and len(si.on_wait) > limit:
                waits = list(si.on_wait)
                excess, keep = waits[:-limit], waits[-limit:]
                for i in range(0, len(excess), limit):
                    n += 1
                    out.append(
                        mybir.InstNoOp(
                            name=f"I-waitsplit-{n}",
                            engine=inst.engine,
                            bass_nofuse=True,
                            sync_info=mybir.SyncInfo(
                                on_wait=excess[i : i + limit], on_update=[]
                            ),
                        )
                    )
                inst.sync_info = mybir.SyncInfo(on_wait=keep, on_update=si.on_update)
                changed = True
            out.append(inst)
        if changed:
            bb.instructions = out
    return n


def _build():
    nc = bass.Bass()
    Vd = nc.declare_dram_parameter("V", [BL, C, XI, H, NJ], BF16, isOutput=False)
    # t = kh*4 + xi
    Ud = nc.declare_dram_parameter("U", [BL, KK * XI, C, C], BF16, isOutput=False)
    Od = nc.declare_dram_parameter("out", [BL, C, 2, HO, NJ], BF16, isOutput=True)

    with TileContext(nc) as tc:
        with (
            tc.tile_pool(name="vt", bufs=2) as p_v,
            tc.tile_pool(name="ut", bufs=2) as p_u,
            tc.tile_pool(name="mt", bufs=12) as p_m,
            tc.tile_pool(name="tt", bufs=6) as p_t,
            tc.tile_pool(name="yt", bufs=6) as p_y,
            tc.tile_pool(name="pacc", bufs=8, space="PSUM") as p_acc,
        ):
            def emit_load(b, nchunks):
                vt = p_v.tile([C, VSZ], BF16, tag="vt")
                step = VSZ // nchunks
                for c0 in range(0, VSZ, step):
                    nc.sync.dma_start(
                        out=vt[:, c0 : c0 + step],
                        in_=Vd[b].rearrange("c x r j -> c (x r j)")[
                            :, c0 : c0 + step
                        ],
                    )
                ut = p_u.tile([C, KK * XI * C], BF16, tag="ut")
                nc.gpsimd.dma_start(
                    out=ut[:, :].rearrange("ci (t co) -> ci t co", t=KK * XI),
                    in_=Ud[b].rearrange("t ci co -> ci t co"),
                )
                return (vt, ut)

            def emit_compute(b, st):
                vt, ut = st
                for g in range(NG):
                    hp0 = HPG * g
                    nh = min(HPG, HO - hp0)
                    n = nh * NJ
                    P = [
                        p_acc.tile([C, 504], F32, tag="P", name=f"P{g}_{i}")
                        for i in range(XI)
                    ]
                    for xi in range(XI):
                        for kh in range(KK):
                            off = xi * H * NJ + (hp0 + kh) * NJ
                            nc.tensor.matmul(
                                P[xi][:, 0:n],
                                ut[:, (kh * XI + xi) * C : (kh * XI + xi + 1) * C],
                                vt[:, off : off + n],
                                start=(kh == 0),
                                stop=(kh == KK - 1),
                            )
                    # Evacuate: ACT takes M0..M2 (3 copies), DVE takes M3;
                    # ScalarE+VectorE may touch psum concurrently on
                    # different banks.
                    m = [
                        p_m.tile([C, 504], BF16, tag="m", name=f"m{g}_{i}")
                        for i in range(XI)
                    ]
                    for xi in range(KK):
                        nc.scalar.copy(m[xi][:, 0:n], P[xi][:, 0:n])
                    nc.vector.tensor_copy(m[3][:, 0:n], P[3][:, 0:n])
                    # Inverse transform (bf16 SBUF, 2x DVE mode):
                    te = p_t.tile([C, 504], BF16, tag="t")
                    to = p_t.tile([C, 504], BF16, tag="t")
                    y = p_y.tile([C, 2 * 504], BF16, tag="y")
                    nc.vector.tensor_add(te[:, 0:n], m[0][:, 0:n], m[1][:, 0:n])
                    nc.vector.tensor_add(y[:, 0:n], te[:, 0:n], m[2][:, 0:n])
                    nc.vector.tensor_sub(to[:, 0:n], m[1][:, 0:n], m[2][:, 0:n])
                    nc.vector.tensor_sub(
                        y[:, n : 2 * n], to[:, 0:n], m[3][:, 0:n]
                    )
                    nc.scalar.dma_start(
                        out=Od[b, :, :, hp0 : hp0 + nh, :],
                        in_=y[:, 0 : 2 * n].rearrange(
                            "c (a h j) -> c a h j", a=2, h=nh
                        ),
                    )

            st = emit_load(0, nchunks=4)
            for b in range(BL):
                nxt = emit_load(b + 1, nchunks=2) if b + 1 < BL else None
                emit_compute(b, st)
                st = nxt

    _split_excess_waits(nc)
    return nc


_CACHE = {}


def _get_nc():
    if "nc" not in _CACHE:
        _CACHE["nc"] = _build()
    return _CACHE["nc"]


def _prep(X, kern):
    """Host-side Winograd F(2,3) input/kernel transforms (bf16)."""
    bf16 = ml_dtypes.bfloat16
    xe = X[:, :, 0::2, :]
    xo = X[:, :, 1::2, :]
    V = np.empty((B, XI, H, NJ, C), np.float32)
    V[:, 0] = xe[:, :, :NJ] - xe[:, :, 1 : NJ + 1]
    V[:, 1] = xo[:, :, :NJ] + xe[:, :, 1 : NJ + 1]
    V[:, 2] = xe[:, :, 1 : NJ + 1] - xo[:, :, :NJ]
    V[:, 3] = xo[:, :, :NJ] - xo[:, :, 1 : NJ + 1]
    Vb = np.ascontiguousarray(V.transpose(0, 4, 1, 2, 3)).astype(bf16)
    G = np.array(
        [[1, 0, 0], [0.5, 0.5, 0.5], [0.5, -0.5, 0.5], [0, 0, 1]], np.float32
    )
    # U[b, kh, xi, ci, co]; t = kh*4+xi
    U = np.einsum("xw,bhwio->bhxio", G, kern)
    Ub = np.ascontiguousarray(U.reshape(B, KK * XI, C, C)).astype(bf16)
    return Vb, Ub


def _run(X, kern, **kw):
    Vb, Ub = _prep(X, kern)
    in_maps = [
        {
            "V": np.ascontiguousarray(Vb[c * BL : (c + 1) * BL]),
            "U": np.ascontiguousarray(Ub[c * BL : (c + 1) * BL]),
        }
        for c in range(N_CORES)
    ]
    last_err = None
    for _attempt in range(3):
        try:
            res = run_bass_kernel_spmd(
                _get_nc(), in_maps, list(range(N_CORES)), **kw
            )
            break
        except Exception as e:  # transient NRT_EXEC_UNIT_UNRECOVERABLE etc.
            last_err = e
    else:
        raise last_err
    # device out: [BL, C, 2, HO, NJ] -> [B, HO, W, C]
    O = np.concatenate(
        [np.asarray(res.results[c]["out"]) for c in range(N_CORES)], axis=0
    ).astype(np.float32)
    out = np.ascontiguousarray(
        O.transpose(0, 3, 4, 2, 1).reshape(B, HO, WO, C)
    )
    return out, res


def kernel(X, kernel):
    X = np.ascontiguousarray(X, dtype=np.float32)
    kern = np.ascontiguousarray(kernel, dtype=np.float32)
    out, _ = _run(X, kern)
    return out


# revision 7
# speedup vs baseline: 1.4225x; 1.1844x over previous
"""Per-sample dynamic conv2d (VALID) on 8 Trainium2 NeuronCores — v4.

v4 = 1-D Winograd F(2,3) along W, direct 3-tap accumulation along H.
Cuts the PE moving-column count 1.5x vs the direct v3 kernel (145k ->
97k streamed columns per sample).

Math per sample (hp = output row, jt = W-tile, w = 2*jt + a):
  V0 = x[2j]-x[2j+2], V1 = x[2j+1]+x[2j+2],
  V2 = x[2j+2]-x[2j+1], V3 = x[2j+1]-x[2j+3]          (host, bf16)
  U[kh,xi] = G @ K[kh,:]  (G = F(2,3) kernel transform) (host, bf16)
  M[xi][hp] = sum_kh V[xi][hp+kh] @ U[kh,xi]            (PE, 12 MMs/group)
  y_even = M0+M1+M2,  y_odd = M1-M2-M3                  (ACT copies + DVE adds)

Device layout: psum/output partitions = Cout.  Output is written as
[C, 2, HO, 63] (even/odd de-interleaved) and the host transposes back —
all device DMA is therefore fully linear.

Per 8-row group x 4 xi: 3 accumulating matmuls (stationary U[kh,xi],
moving = contiguous 504-col V slice) into 4 psum banks; ACT evacuates
M0..M2 to bf16 SBUF, DVE evacuates M3 and does the 4 inverse-transform
adds.  2 groups of psum (8 banks) in flight.
"""

import numpy as np
import ml_dtypes

import concourse.bass as bass
import concourse.mybir as mybir
from concourse.bass_utils import run_bass_kernel_spmd
from concourse.tile import TileContext

N_CORES = 8
B, H, W, C = 32, 128, 128, 128
KK = 3
XI = 4                       # Winograd phases
BL = B // N_CORES            # samples per core
HO = WO = H - KK + 1         # 126
NJ = WO // 2                 # 63 W-tiles
HPG = 8                      # output rows per group
NG = (HO + HPG - 1) // HPG   # 16 groups (last holds 6 rows)
VSZ = XI * H * NJ            # 32256 free elems of V per sample

F32 = mybir.dt.float32
BF16 = mybir.dt.bfloat16


def _split_excess_waits(nc, limit=1):
    """walrus codegen rejects >1 sync-wait on several instruction kinds.
    Move excess waits onto preceding same-engine NoOps."""
    n = 0
    for bb in nc.m.functions[0].blocks:
        out = []
        changed = False
        for inst in bb.instructions:
            si = inst.sync_info
            if si is not None and len(si.on_wait) > limit:
                waits = list(si.on_wait)
                excess, keep = waits[:-limit], waits[-limit:]
                for i in range(0, len(excess), limit):
                    n += 1
                    out.append(
                        mybir.InstNoOp(
                            name=f"I-waitsplit-{n}",
                            engine=inst.engine,
                            bass_nofuse=True,
                            sync_info=mybir.SyncInfo(
                                on_wait=excess[i : i + limit], on_update=[]
                            ),
                        )
                    )
                inst.sync_info = mybir.SyncInfo(on_wait=keep, on_update=si.on_update)
                changed = True
            out.append(inst)
        if changed:
            bb.instructions = out
    return n


def _build():
    nc = bass.Bass()
    Vd = nc.declare_dram_parameter("V", [BL, C, XI, H, NJ], BF16, isOutput=False)
    # t = kh*4 + xi
    Ud = nc.declare_dram_parameter("U", [BL, KK * XI, C, C], BF16, isOutput=False)
    Od = nc.declare_dram_parameter("out", [BL, C, 2, HO, NJ], BF16, isOutput=True)

    with TileContext(nc) as tc:
        with (
            tc.tile_pool(name="vt", bufs=2) as p_v,
            tc.tile_pool(name="ut", bufs=2) as p_u,
            tc.tile_pool(name="mt", bufs=12) as p_m,
            tc.tile_pool(name="tt", bufs=6) as p_t,
            tc.tile_pool(name="yt", bufs=6) as p_y,
            tc.tile_pool(name="pacc", bufs=4, space="PSUM") as p_acc,
        ):
            def emit_load(b, nchunks):
                vt = p_v.tile([C, VSZ], BF16, tag="vt")
                step = VSZ // nchunks
                for c0 in range(0, VSZ, step):
                    nc.sync.dma_start(
                        out=vt[:, c0 : c0 + step],
                        in_=Vd[b].rearrange("c x r j -> c (x r j)")[
                            :, c0 : c0 + step
                        ],
                    )
                ut = p_u.tile([C, KK * XI * C], BF16, tag="ut")
                nc.gpsimd.dma_start(
                    out=ut[:, :].rearrange("ci (t co) -> ci t co", t=KK * XI),
                    in_=Ud[b].rearrange("t ci co -> ci t co"),
                )
                return (vt, ut)

            def emit_compute(b, st):
                vt, ut = st
                yt = None
                for g in range(NG):
                    hp0 = HPG * g
                    nh = min(HPG, HO - hp0)
                    n = nh * NJ
                    # Pair psum banks: P01 holds M0 at [:,0:n] (bank A) and
                    # M1 at [:,512:512+n] (bank B); P23 likewise. ACT then
                    # evacuates each pair in ONE activate (FD=512+n).
                    P01 = p_acc.tile([C, 1024], F32, tag="P", name=f"P01_{g}")
                    P23 = p_acc.tile([C, 1024], F32, tag="P", name=f"P23_{g}")
                    for xi in range(XI):
                        pt = P01 if xi < 2 else P23
                        o0 = (xi % 2) * 512
                        for kh in range(KK):
                            off = xi * H * NJ + (hp0 + kh) * NJ
                            nc.tensor.matmul(
                                pt[:, o0 : o0 + n],
                                ut[:, (kh * XI + xi) * C : (kh * XI + xi + 1) * C],
                                vt[:, off : off + n],
                                start=(kh == 0),
                                stop=(kh == KK - 1),
                            )
                    # ACT evacuates both psum pairs (bf16); DVE does only the
                    # inverse-transform adds (bf16 SBUF, 2x DVE mode).
                    m01 = p_m.tile([C, 1024], BF16, tag="m", name=f"m01_{g}")
                    m23 = p_m.tile([C, 1024], BF16, tag="m", name=f"m23_{g}")
                    nc.scalar.copy(m01[:, 0 : 512 + n], P01[:, 0 : 512 + n])
                    nc.scalar.copy(m23[:, 0 : 512 + n], P23[:, 0 : 512 + n])
                    m0, m1 = m01[:, 0:n], m01[:, 512 : 512 + n]
                    m2, m3 = m23[:, 0:n], m23[:, 512 : 512 + n]
                    te = p_t.tile([C, 504], BF16, tag="t")
                    to = p_t.tile([C, 504], BF16, tag="t")
                    yt = p_y.tile([C, 2 * 504], BF16, tag="y")
                    nc.vector.tensor_add(te[:, 0:n], m0, m1)
                    nc.vector.tensor_add(yt[:, 0:n], te[:, 0:n], m2)
                    nc.vector.tensor_sub(to[:, 0:n], m1, m2)
                    nc.vector.tensor_sub(yt[:, n : 2 * n], to[:, 0:n], m3)
                    # Store on the (otherwise idle) Sync ring.
                    nc.sync.dma_start(
                        out=Od[b, :, :, hp0 : hp0 + nh, :],
                        in_=yt[:, 0 : 2 * n].rearrange(
                            "c (a h j) -> c a h j", a=2, h=nh
                        ),
                    )

            st = emit_load(0, nchunks=4)
            for b in range(BL):
                nxt = emit_load(b + 1, nchunks=2) if b + 1 < BL else None
                emit_compute(b, st)
                st = nxt

    _split_excess_waits(nc)
    return nc


_CACHE = {}


def _get_nc():
    if "nc" not in _CACHE:
        _CACHE["nc"] = _build()
    return _CACHE["nc"]


def _prep(X, kern):
    """Host-side Winograd F(2,3) input/kernel transforms (bf16)."""
    bf16 = ml_dtypes.bfloat16
    xe = X[:, :, 0::2, :]
    xo = X[:, :, 1::2, :]
    V = np.empty((B, XI, H, NJ, C), np.float32)
    V[:, 0] = xe[:, :, :NJ] - xe[:, :, 1 : NJ + 1]
    V[:, 1] = xo[:, :, :NJ] + xe[:, :, 1 : NJ + 1]
    V[:, 2] = xe[:, :, 1 : NJ + 1] - xo[:, :, :NJ]
    V[:, 3] = xo[:, :, :NJ] - xo[:, :, 1 : NJ + 1]
    Vb = np.ascontiguousarray(V.transpose(0, 4, 1, 2, 3)).astype(bf16)
    G = np.array(
        [[1, 0, 0], [0.5, 0.5, 0.5], [0.5, -0.5, 0.5], [0, 0, 1]], np.float32
    )
    # U[b, kh, xi, ci, co]; t = kh*4+xi
    U = np.einsum("xw,bhwio->bhxio", G, kern)
    Ub = np.ascontiguousarray(U.reshape(B, KK * XI, C, C)).astype(bf16)
    return Vb, Ub


def _run(X, kern, **kw):
    Vb, Ub = _prep(X, kern)
    in_maps = [
        {
            "V": np.ascontiguousarray(Vb[c * BL : (c + 1) * BL]),
            "U": np.ascontiguousarray(Ub[c * BL : (c + 1) * BL]),
        }
        for c in range(N_CORES)
    ]
    last_err = None
    for _attempt in range(3):
        try:
            res = run_bass_kernel_spmd(
                _get_nc(), in_maps, list(range(N_CORES)), **kw
            )
            break
        except Exception as e:  # transient NRT_EXEC_UNIT_UNRECOVERABLE etc.
            last_err = e
    else:
        raise last_err
    # device out: [BL, C, 2, HO, NJ] -> [B, HO, W, C]
    O = np.concatenate(
        [np.asarray(res.results[c]["out"]) for c in range(N_CORES)], axis=0
    ).astype(np.float32)
    out = np.ascontiguousarray(
        O.transpose(0, 3, 4, 2, 1).reshape(B, HO, WO, C)
    )
    return out, res


def kernel(X, kernel):
    X = np.ascontiguousarray(X, dtype=np.float32)
    kern = np.ascontiguousarray(kernel, dtype=np.float32)
    out, _ = _run(X, kern)
    return out
